# revision 6
# baseline (speedup 1.0000x reference)
"""4D multilinear interpolation (8x8x8x8 lattice) on 8 Trainium2 cores.

For each row b: scale coordinates[b] (4 values in [0,1)) to the 7-cell
lattice, find the containing cell, gather the 16 corner values from
mesh_pred[b] (4096 values), and blend with multilinear weights.

Gather strategy: per-row indirect DMA (one descriptor per row via
gpsimd.indirect_dma_start) costs 994ns of SWDGE fixed overhead per
instruction and only does 128 rows per instruction (one offset per
partition) -> 32 instructions ~ 45us serial on GpSimd.  Instead use
gpsimd.dma_gather (InstDMAGatherAnt): int16 chunk indices, thousands of
descriptors per instruction (994 + 0.34ns/desc).  Each row fetches TWO
128-float chunk-aligned windows (the a=0 / a=1 hyperplanes of lattice
dim 0, window start 64*(8a+b+8da)), so all 16 corners of plane da sit at
in-window offset o + 64*db + 8*dc + dd with o = 8*ci2+ci3 <= 54.
8 instructions x 1024 descriptors of 512B = 4.2MB/core.

The data-dependent in-window offset o is absorbed by the blend: the
(dim2,dim3) bilinear reduction is a dot over t in [0,64) against a dense
separable weight vector W64[8u+v] = hat_u(c2)*hat_v(c3), where
hat_k(x) = relu(1-|k-x|) needs no floor at all.  Dims 0,1 contribute the
4 plane weights wab = (da?f0:1-f0)*(db?f1:1-f1).

Row layout: device slot (p, n) holds DRAM row ord = 512*(n//4) +
128*(n%4) + p (host permutes coordinates/output accordingly; mesh_pred
is passed through unpermuted).  dma_gather instruction g covers DRAM
rows [512g, 512g+512): gathered position j = jh*128+p lands at
out[p, jh]; idx value at wrapped position j (= col*16 + partition%16)
is ord_local*64 + (8a+b) + 8*plane.
"""

import numpy as np

import concourse.bass as bass
import concourse.bacc as bacc
import concourse.mybir as mybir
from concourse import bass_utils
from concourse.tile import TileContext

P = 128          # partitions
I = 32           # row slots per partition
BC = P * I       # 4096 rows per core
VOL = 4096       # 8^4 lattice values per row
ND = 4
NCORES = 8
MESH = 8
NG = 8           # dma_gather instructions per core
RPI = BC // NG   # rows per gather instruction (512)
NIDX = 2 * RPI   # indices per gather instruction (2 windows/row)
ES = 128         # elem_size (floats) per gathered window
EST = 64         # elem_step (floats): 256B index granularity
F32 = mybir.dt.float32
I16 = mybir.dt.int16
OP = mybir.AluOpType


def _build():
    nc = bacc.Bacc("TRN2", target_bir_lowering=False, debug=False)
    coords = nc.dram_tensor("coordinates", [BC, ND], F32, kind="ExternalInput")
    mesh = nc.dram_tensor("mesh_pred", [BC, VOL], F32, kind="ExternalInput")
    # dims 0,1 of coords replicated in the dma_gather wrapped-index layout:
    # [16c+w, g*64 + 2m + dim] = coords[512g + 16m + w, dim]
    coords2 = nc.dram_tensor("coords2", [P, NG * 64], F32, kind="ExternalInput")
    # (16m + w)*64 at [16c+w, g*32 + m]
    jbase = nc.dram_tensor("jbase", [P, NG * 32], F32, kind="ExternalInput")
    # u at [p, u]
    ugrid = nc.dram_tensor("ugrid", [P, 8], F32, kind="ExternalInput")
    out = nc.dram_tensor("out", [BC], F32, kind="ExternalOutput")

    out_t = out[:].rearrange("(p n) -> p n", p=P)
    coords_t = coords[:].rearrange("(p n) d -> p (n d)", p=P)

    with TileContext(nc) as tc:
        with tc.tile_pool(name="pool", bufs=1) as pool:
            c2t = pool.tile([P, NG * 64], F32, tag="c2t")
            nc.sync.dma_start(out=c2t[:], in_=coords2[:])
            ct = pool.tile([P, I * ND], F32, tag="ct")
            nc.sync.dma_start(out=ct[:], in_=coords_t)
            jb = pool.tile([P, NG * 32], F32, tag="jb")
            nc.sync.dma_start(out=jb[:], in_=jbase[:])
            ug = pool.tile([P, 8], F32, tag="ug")
            nc.sync.dma_start(out=ug[:], in_=ugrid[:])

            # ---- int16 gather indices (wrapped layout) ----
            cc2 = pool.tile([P, NG * 64], F32, tag="cc2")
            nc.vector.tensor_scalar_mul(cc2[:], c2t[:], float(MESH - 1))
            ges = []
            for k in range(1, MESH - 1):
                gk = pool.tile([P, NG * 64], F32, tag=f"g2_{k}")
                nc.vector.tensor_scalar(
                    out=gk[:], in0=cc2[:], scalar1=float(k), scalar2=None,
                    op0=OP.is_ge,
                )
                ges.append(gk)
            while len(ges) > 1:
                nxt = []
                for a in range(0, len(ges) - 1, 2):
                    sm = pool.tile([P, NG * 64], F32, tag=f"gs2_{len(ges)}_{a}")
                    nc.vector.tensor_tensor(
                        out=sm[:], in0=ges[a][:], in1=ges[a + 1][:], op=OP.add
                    )
                    nxt.append(sm)
                if len(ges) % 2:
                    nxt.append(ges[-1])
                ges = nxt
            i2 = ges[0]
            t8 = pool.tile([P, NG * 32], F32, tag="t8")
            nc.vector.tensor_scalar_mul(t8[:], i2[:, 0::2], 8.0)
            tb = pool.tile([P, NG * 32], F32, tag="tb")
            nc.vector.tensor_tensor(out=tb[:], in0=t8[:], in1=i2[:, 1::2], op=OP.add)
            v0 = pool.tile([P, NG * 32], F32, tag="v0")
            nc.vector.tensor_tensor(out=v0[:], in0=tb[:], in1=jb[:], op=OP.add)
            v1 = pool.tile([P, NG * 32], F32, tag="v1")
            nc.vector.tensor_scalar(
                out=v1[:], in0=v0[:], scalar1=8.0, scalar2=None, op0=OP.add
            )
            idx16 = pool.tile([P, NG * 64], I16, tag="idx16")
            ix = idx16[:]
            # plane-0 cols [g*64, g*64+32), plane-1 cols [g*64+32, g*64+64)
            dst0 = bass.AP(ix.tensor, ix.offset, [ix.ap[0], [64, NG], [1, 32]])
            dst1 = bass.AP(ix.tensor, ix.offset + 32, [ix.ap[0], [64, NG], [1, 32]])
            s0 = v0[:]
            src0 = bass.AP(s0.tensor, s0.offset, [s0.ap[0], [32, NG], [1, 32]])
            s1 = v1[:]
            src1 = bass.AP(s1.tensor, s1.offset, [s1.ap[0], [32, NG], [1, 32]])
            nc.vector.tensor_copy(out=dst0, in_=src0)
            nc.vector.tensor_copy(out=dst1, in_=src1)

            # ---- 8 multi-descriptor gathers ----
            G = pool.tile([P, NG * 8 * ES], F32, tag="G")
            gb = G[:]
            mesh_ap = mesh[:]
            for g in range(NG):
                inap = bass.AP(
                    mesh_ap.tensor,
                    g * RPI * VOL,
                    [[EST, RPI * VOL // EST - 1], [1, ES]],
                )
                gdst = G[:, g * 8 * ES:(g + 1) * 8 * ES].rearrange(
                    "p (j t) -> p j t", t=ES
                )
                nc.gpsimd.dma_gather(
                    gdst,
                    inap,
                    idx16[:, g * 64:(g + 1) * 64],
                    NIDX,
                    NIDX,
                    ES,
                    elem_step=EST,
                )

            # ---- weights ----
            c = pool.tile([P, I * ND], F32, tag="c")
            nc.vector.tensor_scalar_mul(c[:], ct[:], float(MESH - 1))
            # c01[p, n*2+d], d in {0,1}: strided copy of dims 0,1
            ca = c[:]
            c01v = bass.AP(ca.tensor, ca.offset, [ca.ap[0], [ND, I], [1, 2]])
            ges = []
            for k in range(1, MESH - 1):
                gk = pool.tile([P, I * 2], F32, tag=f"g01_{k}")
                nc.vector.tensor_scalar(
                    out=gk[:], in0=c01v, scalar1=float(k), scalar2=None,
                    op0=OP.is_ge,
                )
                ges.append(gk)
            while len(ges) > 1:
                nxt = []
                for a in range(0, len(ges) - 1, 2):
                    sm = pool.tile([P, I * 2], F32, tag=f"gs01_{len(ges)}_{a}")
                    nc.vector.tensor_tensor(
                        out=sm[:], in0=ges[a][:], in1=ges[a + 1][:], op=OP.add
                    )
                    nxt.append(sm)
                if len(ges) % 2:
                    nxt.append(ges[-1])
                ges = nxt
            ci01 = ges[0]
            fr = pool.tile([P, I * 2], F32, tag="fr")
            nc.vector.tensor_tensor(out=fr[:], in0=c01v, in1=ci01[:], op=OP.subtract)
            om = pool.tile([P, I * 2], F32, tag="om")
            nc.vector.tensor_scalar(
                out=om[:], in0=fr[:], scalar1=-1.0, scalar2=1.0,
                op0=OP.mult, op1=OP.add,
            )
            # wab[p, ab*I + n]
            wab = pool.tile([P, 4 * I], F32, tag="wab")
            for ab in range(4):
                da, db = ab >> 1, ab & 1
                nc.vector.tensor_tensor(
                    out=wab[:, ab * I:(ab + 1) * I],
                    in0=(fr if da else om)[:, 0::2],
                    in1=(fr if db else om)[:, 1::2], op=OP.mult,
                )

            # hat grids for dims 2,3: w[p, n*8+u] = relu(1 - |u - c_dim|)
            AF = mybir.ActivationFunctionType

            def hat(dim, tag):
                d = pool.tile([P, I * 8], F32, tag=f"{tag}d")
                ua = ug[:]
                uv = bass.AP(ua.tensor, ua.offset, [ua.ap[0], [0, I], [1, 8]])
                cv = bass.AP(ca.tensor, ca.offset + dim, [ca.ap[0], [ND, I], [0, 8]])
                nc.vector.tensor_tensor(out=d[:], in0=uv, in1=cv, op=OP.subtract)
                a = pool.tile([P, I * 8], F32, tag=f"{tag}a")
                nc.scalar.activation(a[:], d[:], AF.Abs)
                w = pool.tile([P, I * 8], F32, tag=f"{tag}w")
                nc.scalar.activation(w[:], a[:], AF.Relu, bias=1.0, scale=-1.0)
                return w

            wu = hat(2, "wu")
            wv = hat(3, "wv")

            # ---- blend, chunked over halves of the gather instructions ----
            acc = pool.tile([P, I], F32, tag="acc")
            NH = NG // 2  # nhi blocks per chunk
            for h in range(2):
                nh0 = h * NH
                S = pool.tile([P, NH * 4 * 64], F32, tag=f"S{h}")
                sv = S[:]
                for ab in range(4):
                    da, db = ab >> 1, ab & 1
                    gv = bass.AP(
                        gb.tensor,
                        gb.offset + nh0 * 1024 + da * 512 + db * 64,
                        [gb.ap[0], [1024, NH], [ES, 4], [1, 64]],
                    )
                    wv_ab = wab[:]
                    wv_b = bass.AP(
                        wv_ab.tensor,
                        wv_ab.offset + ab * I + nh0 * 4,
                        [wv_ab.ap[0], [4, NH], [1, 4], [0, 64]],
                    )
                    sdst = bass.AP(
                        sv.tensor, sv.offset,
                        [sv.ap[0], [256, NH], [64, 4], [1, 64]],
                    )
                    if ab == 0:
                        nc.vector.tensor_tensor(out=sdst, in0=gv, in1=wv_b, op=OP.mult)
                    else:
                        T = pool.tile([P, NH * 4 * 64], F32, tag=f"T{h}_{ab}")
                        tv = T[:]
                        tdst = bass.AP(
                            tv.tensor, tv.offset,
                            [tv.ap[0], [256, NH], [64, 4], [1, 64]],
                        )
                        nc.vector.tensor_tensor(out=tdst, in0=gv, in1=wv_b, op=OP.mult)
                        nc.vector.tensor_tensor(out=S[:], in0=S[:], in1=T[:], op=OP.add)

                # contract v: P2 = S * wv  (S cols n*64 + 8u + v, n local)
                nrows = NH * 4  # 16 rows per chunk
                P2 = pool.tile([P, nrows * 64], F32, tag=f"P2{h}")
                p2v = P2[:]
                wvv = wv[:]
                nc.vector.tensor_tensor(
                    out=bass.AP(p2v.tensor, p2v.offset,
                                [p2v.ap[0], [64, nrows], [8, 8], [1, 8]]),
                    in0=bass.AP(sv.tensor, sv.offset,
                                [sv.ap[0], [64, nrows], [8, 8], [1, 8]]),
                    in1=bass.AP(wvv.tensor, wvv.offset + nh0 * 4 * 8,
                                [wvv.ap[0], [8, nrows], [0, 8], [1, 8]]),
                    op=OP.mult,
                )
                Y = pool.tile([P, nrows * 8], F32, tag=f"Y{h}")
                nc.vector.tensor_reduce(
                    out=Y[:].rearrange("p (r u) -> p r u", u=8),
                    in_=bass.AP(p2v.tensor, p2v.offset,
                                [p2v.ap[0], [8, nrows * 8], [1, 8]]),
                    axis=mybir.AxisListType.X,
                    op=OP.add,
                )
                # contract u: P3 = Y * wu  (cols n*8+u)
                P3 = pool.tile([P, nrows * 8], F32, tag=f"P3{h}")
                wuv = wu[:]
                nc.vector.tensor_tensor(
                    out=P3[:],
                    in0=Y[:],
                    in1=bass.AP(wuv.tensor, wuv.offset + nh0 * 4 * 8,
                                [wuv.ap[0], [1, nrows * 8]]),
                    op=OP.mult,
                )
                p3v = P3[:]
                nc.vector.tensor_reduce(
                    out=acc[:, nh0 * 4:(nh0 + NH) * 4].rearrange(
                        "p (r o) -> p r o", o=1
                    ),
                    in_=bass.AP(p3v.tensor, p3v.offset,
                                [p3v.ap[0], [8, nrows], [1, 8]]),
                    axis=mybir.AxisListType.X,
                    op=OP.add,
                )

            nc.sync.dma_start(out=out_t, in_=acc[:])
    nc.compile()
    return nc


_NC = None


def _get_nc():
    global _NC
    if _NC is None:
        _NC = _build()
    return _NC


# slot (p, n) <-> DRAM row ord = 512*(n//4) + 128*(n%4) + p
_n = np.arange(I)
_ORD = (512 * (_n[None, :] // 4) + 128 * (_n[None, :] % 4)
        + np.arange(P)[:, None]).reshape(-1)  # [p*I + n]

# coords2 source rows: [w, g, m] -> 512g + 16m + w
_w = np.arange(16)
_g = np.arange(NG)
_m = np.arange(32)
_ORD2 = (512 * _g[None, :, None] + 16 * _m[None, None, :]
         + _w[:, None, None])  # [16, 8, 32]

_JB = np.broadcast_to(
    ((16 * _m[None, None, :] + _w[:, None, None]) * 64).astype(np.float32),
    (16, NG, 32),
).reshape(16, NG * 32)
_JB128 = np.ascontiguousarray(np.tile(_JB, (8, 1)))  # [128, 256]
_UG = np.ascontiguousarray(
    np.broadcast_to(np.arange(8, dtype=np.float32)[None, :], (P, 8))
)


def kernel(coordinates, mesh_pred, _trace=False, _tmpdir=None):
    coordinates = np.asarray(coordinates, dtype=np.float32)
    mesh_pred = np.asarray(mesh_pred, dtype=np.float32)
    assert coordinates.shape == (NCORES * BC, ND)
    assert mesh_pred.shape == (NCORES * BC, VOL)

    in_maps = []
    for cix in range(NCORES):
        sl = slice(cix * BC, (cix + 1) * BC)
        cs = coordinates[sl]
        cs_perm = np.ascontiguousarray(cs[_ORD])
        c2 = cs[_ORD2][:, :, :, :2]  # [16, 8, 32, 2]
        c2 = np.ascontiguousarray(
            np.tile(c2.reshape(16, NG * 64), (8, 1))
        )  # [128, 512]
        in_maps.append(
            {
                "coordinates": cs_perm,
                "mesh_pred": np.ascontiguousarray(mesh_pred[sl]),
                "coords2": c2,
                "jbase": _JB128,
                "ugrid": _UG,
            }
        )
    res = bass_utils.run_bass_kernel_spmd(
        _get_nc(),
        in_maps,
        core_ids=list(range(NCORES)),
        trace=_trace,
        tmpdir=_tmpdir,
    )
    outs = []
    for r in res.results:
        o = np.asarray(r["out"]).reshape(-1)  # [p*I + n]
        full = np.empty(BC, dtype=np.float32)
        full[_ORD] = o
        outs.append(full)
    out = np.concatenate(outs)
    if _trace:
        return out, res
    return out


# revision 8
# speedup vs baseline: 1.6936x; 1.6936x over previous
"""4D multilinear interpolation (8x8x8x8 lattice) on 8 Trainium2 cores.

For each row b: scale coordinates[b] (4 values in [0,1)) to the 7-cell
lattice, find the containing cell, gather the 16 corner values from
mesh_pred[b] (4096 values), and blend with multilinear weights.

HW constraint (measured): indirect DMA gather consumes ONE index per
partition and streams the dest free-width contiguously from it.  So rows
are laid out b = n*128 + p (host pre-permutes coordinates into (p,n)
order; output is permuted back) and each of the 32 gathers fetches, per
partition, the 586-float span that covers all 16 cell corners of one row.
Corner extraction is then a fixed multi-dim strided view ([512,2],[64,2],
[8,2],[1,2]) of the gathered span; the weighted blend runs as a handful
of wide DVE ops instead of per-row arithmetic.
"""

import numpy as np

import concourse.bass as bass
import concourse.bacc as bacc
import concourse.mybir as mybir
from concourse import bass_utils
from concourse.tile import TileContext

P = 128          # partitions
I = 32           # row-tiles (gathers) per core
BC = P * I       # 4096 rows per core
VOL = 4096       # 8^4 lattice values per row
ND = 4
NCORES = 8
MESH = 8
SPANW = 640      # padded per-row gather width (586 used)
SPAN = 586       # 585 max corner offset + 1
F32 = mybir.dt.float32
I32 = mybir.dt.int32
OP = mybir.AluOpType


def _build():
    nc = bacc.Bacc("TRN2", target_bir_lowering=False, debug=False)
    # coordinates arrive host-permuted: device row p*I+n = original row n*P+p
    coords = nc.dram_tensor("coordinates", [BC, ND], F32, kind="ExternalInput")
    mesh = nc.dram_tensor("mesh_pred", [BC, VOL], F32, kind="ExternalInput")
    out = nc.dram_tensor("out", [BC], F32, kind="ExternalOutput")

    mesh_2d = mesh[:]
    coords_t = coords[:].rearrange("(p n) d -> p (n d)", p=P)
    out_t = out[:].rearrange("(p n) -> p n", p=P)  # host permutes back

    with TileContext(nc) as tc:
        with tc.tile_pool(name="pool", bufs=1) as pool:
            ct = pool.tile([P, I * ND], F32, tag="ct")
            nc.sync.dma_start(out=ct[:], in_=coords_t)

            # flat row base for original row n*P+p: (n*P+p)*VOL
            # iota pattern steps are int16-limited, so generate n*P+p and
            # shift left by log2(VOL) on DVE (also absorbs the Pool sem)
            tbl = pool.tile([P, I], I32, tag="tbl")
            nc.gpsimd.iota(tbl[:], pattern=[[P, I]], base=0, channel_multiplier=1)
            c = pool.tile([P, I * ND], F32, tag="c")
            nc.vector.tensor_scalar_mul(c[:], ct[:], float(MESH - 1))
            tbl2 = pool.tile([P, I], I32, tag="tbl2")
            nc.vector.tensor_scalar(
                out=tbl2[:], in0=tbl[:], scalar1=12, scalar2=None,
                op0=OP.logical_shift_left,
            )

            # --- per-half idx computation + gather issue (halves the time
            # to first gather: gathers 0-15 start while the idx chain for
            # tiles 16-31 is still running on DVE) ---
            H = I // 2          # tiles per half
            HW = H * ND         # op width per half
            Gbig = pool.tile([P, I * SPANW], F32, tag="Gbig")
            cifs = []
            for h in range(2):
                csl = c[:, h * HW:(h + 1) * HW]
                ges = []
                for k in range(1, MESH - 1):
                    g = pool.tile([P, HW], F32, tag=f"ge{h}_{k}")
                    nc.vector.tensor_scalar(
                        out=g[:], in0=csl, scalar1=float(k), scalar2=None,
                        op0=OP.is_ge,
                    )
                    ges.append(g)
                while len(ges) > 1:
                    nxt = []
                    for a in range(0, len(ges) - 1, 2):
                        s = pool.tile([P, HW], F32, tag=f"gs{h}_{len(ges)}_{a}")
                        nc.vector.tensor_tensor(
                            out=s[:], in0=ges[a][:], in1=ges[a + 1][:], op=OP.add
                        )
                        nxt.append(s)
                    if len(ges) % 2:
                        nxt.append(ges[-1])
                    ges = nxt
                cif = ges[0]
                cifs.append(cif)

                # idx = sum_d cif_d * coef_d + row_base (exact in f32)
                cc = pool.tile([P, HW], F32, tag=f"cc{h}")
                for d, coef in enumerate((512.0, 64.0, 8.0, 1.0)):
                    nc.vector.tensor_scalar_mul(
                        cc[:, d * H:(d + 1) * H], cif[:, d::ND], coef
                    )
                idxf = pool.tile([P, H], F32, tag=f"idxf{h}")
                ccv = cc[:]
                nc.vector.tensor_reduce(
                    out=idxf[:].rearrange("p (n o) -> p n o", o=1),
                    in_=bass.AP(ccv.tensor, ccv.offset,
                                [ccv.ap[0], [1, H], [H, ND]]),
                    axis=mybir.AxisListType.X,
                    op=OP.add,
                )
                idxi = pool.tile([P, H], I32, tag=f"idxi{h}")
                nc.vector.tensor_copy(out=idxi[:], in_=idxf[:])
                idx = pool.tile([P, H], I32, tag=f"idx{h}")
                nc.vector.tensor_tensor(
                    out=idx[:], in0=idxi[:], in1=tbl2[:, h * H:(h + 1) * H],
                    op=OP.add,
                )
                for n in range(H):
                    nt = h * H + n
                    nc.gpsimd.indirect_dma_start(
                        out=Gbig[:, nt * SPANW:nt * SPANW + SPAN],
                        out_offset=None,
                        in_=mesh_2d,
                        in_offset=bass.IndirectOffsetOnAxis(ap=idx[:, n:n + 1], axis=1),
                        element_offset=0,
                    )

            # --- weights (runs on DVE while gathers stream) ---
            frac = pool.tile([P, I * ND], F32, tag="frac")
            for h in range(2):
                nc.vector.tensor_tensor(
                    out=frac[:, h * HW:(h + 1) * HW],
                    in0=c[:, h * HW:(h + 1) * HW], in1=cifs[h][:], op=OP.subtract,
                )
            om = pool.tile([P, I * ND], F32, tag="om")
            nc.vector.tensor_scalar(
                out=om[:], in0=frac[:], scalar1=-1.0, scalar2=1.0,
                op0=OP.mult, op1=OP.add,
            )
            w01 = pool.tile([P, 4 * I], F32, tag="w01")
            w23 = pool.tile([P, 4 * I], F32, tag="w23")
            pairs = ((0, 0), (0, 1), (1, 0), (1, 1))
            for g, (a, b) in enumerate(pairs):
                nc.vector.tensor_tensor(
                    out=w23[:, g * I:(g + 1) * I],
                    in0=(frac if a else om)[:, 2::ND],
                    in1=(frac if b else om)[:, 3::ND], op=OP.mult,
                )
            for g, (a, b) in enumerate(pairs):
                nc.vector.tensor_tensor(
                    out=w01[:, g * I:(g + 1) * I],
                    in0=(frac if a else om)[:, 0::ND],
                    in1=(frac if b else om)[:, 1::ND], op=OP.mult,
                )
            W16 = pool.tile([P, I * 16], F32, tag="W16")  # layout (n, k) k fastest
            for k in range(16):
                g, j = k >> 2, k & 3
                nc.vector.tensor_tensor(
                    out=W16[:, k::16],
                    in0=w01[:, g * I:(g + 1) * I],
                    in1=w23[:, j * I:(j + 1) * I], op=OP.mult,
                )

            # --- blend, per half so the first half overlaps the second
            # half's gather transfers ---
            acc = pool.tile([P, I], F32, tag="acc")
            W16v = W16[:].rearrange("p (n k) -> p n k", k=16)
            for h in range(2):
                M = []
                for ab in range(4):
                    a, b = ab >> 1, ab & 1
                    goff = h * H * SPANW + a * 512 + b * 64
                    gview = Gbig[:]
                    gview = bass.AP(
                        gview.tensor,
                        gview.offset + goff,
                        [gview.ap[0], [SPANW, H], [8, 2], [1, 2]],
                    )
                    wview = bass.AP(
                        W16v.tensor,
                        W16v.offset + h * H * 16 + ab * 4,
                        [W16v.ap[0], [16, H], [2, 2], [1, 2]],
                    )
                    m = pool.tile([P, H * 4], F32, tag=f"M{h}_{ab}")
                    nc.vector.tensor_tensor(
                        out=m[:].rearrange("p (n c d) -> p n c d", c=2, d=2),
                        in0=gview, in1=wview, op=OP.mult,
                    )
                    M.append(m)
                m01 = pool.tile([P, H * 4], F32, tag=f"m01_{h}")
                m23 = pool.tile([P, H * 4], F32, tag=f"m23_{h}")
                msum = pool.tile([P, H * 4], F32, tag=f"msum_{h}")
                nc.vector.tensor_tensor(out=m01[:], in0=M[0][:], in1=M[1][:], op=OP.add)
                nc.vector.tensor_tensor(out=m23[:], in0=M[2][:], in1=M[3][:], op=OP.add)
                nc.vector.tensor_tensor(out=msum[:], in0=m01[:], in1=m23[:], op=OP.add)
                msv = msum[:]
                nc.vector.tensor_reduce(
                    out=acc[:, h * H:(h + 1) * H].rearrange("p (n o) -> p n o", o=1),
                    in_=bass.AP(msv.tensor, msv.offset,
                                [msv.ap[0], [4, H], [1, 4]]),
                    axis=mybir.AxisListType.X,
                    op=OP.add,
                )

            nc.sync.dma_start(out=out_t, in_=acc[:])
    nc.compile()
    return nc


_NC = None


def _get_nc():
    global _NC
    if _NC is None:
        _NC = _build()
    return _NC


def kernel(coordinates, mesh_pred, _trace=False, _tmpdir=None):
    coordinates = np.asarray(coordinates, dtype=np.float32)
    mesh_pred = np.asarray(mesh_pred, dtype=np.float32)
    assert coordinates.shape == (NCORES * BC, ND)
    assert mesh_pred.shape == (NCORES * BC, VOL)

    in_maps = []
    for cix in range(NCORES):
        sl = slice(cix * BC, (cix + 1) * BC)
        cs = coordinates[sl]
        # device row p*I+n must hold original row n*P+p
        cs_perm = np.ascontiguousarray(
            cs.reshape(I, P, ND).transpose(1, 0, 2).reshape(BC, ND)
        )
        in_maps.append(
            {
                "coordinates": cs_perm,
                "mesh_pred": np.ascontiguousarray(mesh_pred[sl]),
            }
        )
    res = bass_utils.run_bass_kernel_spmd(
        _get_nc(),
        in_maps,
        core_ids=list(range(NCORES)),
        trace=_trace,
        tmpdir=_tmpdir,
    )
    outs = []
    for r in res.results:
        o = np.asarray(r["out"]).reshape(P, I)  # [p, n]
        outs.append(o.transpose(1, 0).reshape(-1))  # back to b = n*P+p
    out = np.concatenate(outs)
    if _trace:
        return out, res
    return out

